# revision 1
# baseline (speedup 1.0000x reference)
"""GCNConv (gnn_message_passing) on 8 Trainium2 NeuronCores.

out = D^{-1/2} (A + I) D^{-1/2} (X W) + b

Key identity: with h' = dinv * (X @ W)  (per-node scale),
  out[d] = dinv[d] * ( sum_{e: dst=d} h'[src_e] + h'[d] ) + b
so no per-edge scaling is needed anywhere (self loops are appended as
ordinary edges).

Device plan (SPMD, one program, 8 cores):
  Phase A: core c computes h' for its 12500-node shard:
           x_shard @ W via PE (PE-transpose of x blocks), scale by dinv.
           AllGather -> h_all [8*(shard+1), 64] in every core's DRAM
           (each shard contributes one trailing zero row used as gather pad).
  Phase B: edges partitioned by dst shard, sorted by (dst window, half).
           Per 128-edge slot group: indirect DMA gather of h'[src] rows
           (one int32 row offset per partition), then PE matmul with a
           one-hot "segment" stationary [128 slots, 64 dst cols] built on
           DVE via is_equal(iota, dstoff), accumulated into
           PSUM[64h:64h+64, :] per 128-dst window.  Scale by dinv[dst],
           DMA out.  b added on host.

All slot counts are static and identical across cores (max over cores,
padded with gathers of the zero row), so one NEFF serves all 8 cores.
"""

import numpy as np

P = 128        # partitions
COUT = 64      # output features
HALF = 64      # dst columns per matmul half-window


def _cdiv(a, b):
    return -(-a // b)


# ----------------------------------------------------------------------------
# CPU planning: edge partitioning, slot assignment, offset/dstoff tables
# ----------------------------------------------------------------------------
def _plan(edge_index, N, ncores):
    shard = N // ncores                     # dst nodes per core
    sp = shard + 1                          # +1 zero row per shard in h_all
    nwin = _cdiv(shard, P)                  # dst windows per core

    src = np.asarray(edge_index[0], dtype=np.int64)
    dst = np.asarray(edge_index[1], dtype=np.int64)
    deg = np.bincount(dst, minlength=N).astype(np.float64) + 1.0
    dinv = (1.0 / np.sqrt(deg)).astype(np.float32)

    # append self loops as ordinary edges
    loop = np.arange(N, dtype=np.int64)
    src = np.concatenate([src, loop])
    dst = np.concatenate([dst, loop])

    per_core = []
    counts = np.zeros((ncores, nwin, 2), np.int64)
    for c in range(ncores):
        m = (dst // shard) == c
        s = src[m]
        d = dst[m] - c * shard
        w = d >> 7
        h = (d >> 6) & 1
        grow = s + (s // shard)             # h_all global row (skips 0-rows)
        order = np.lexsort((d, h, w))
        s, d, w, h, grow = (a[order] for a in (s, d, w, h, grow))
        np.add.at(counts[c], (w, h), 1)
        per_core.append((w, h, grow, d))

    # static group counts per (window, half): max over cores
    G = _cdiv(counts.max(axis=0), P)            # [nwin, 2]
    Gflat = G.reshape(-1)
    cellbase = np.concatenate([[0], np.cumsum(Gflat * P)])
    GT = int(Gflat.sum())
    slots = GT * P

    goff = np.empty((ncores, P, GT), np.int32)
    dstoff = np.empty((ncores, P, GT), np.float32)
    for c in range(ncores):
        w, h, grow, d = per_core[c]
        key = w * 2 + h
        start = np.zeros(len(key), bool)
        start[0] = True
        start[1:] = key[1:] != key[:-1]
        runstart = np.flatnonzero(start)
        cum = np.arange(len(key)) - np.repeat(runstart, np.diff(
            np.concatenate([runstart, [len(key)]])))
        slot = cellbase[key] + cum
        g = np.full(slots, shard, np.int64)     # pad: zero row of shard 0
        g[slot] = grow
        doff = np.full(slots, -1.0, np.float32)
        doff[slot] = (d - (w << 7) - (h << 6)).astype(np.float32)
        goff[c] = g.astype(np.int32).reshape(GT, P).T
        dstoff[c] = doff.reshape(GT, P).T

    return dict(shard=shard, sp=sp, nwin=nwin, G=G, GT=GT, dinv=dinv,
                goff=goff, dstoff=dstoff)


# ----------------------------------------------------------------------------
# Device program (one SPMD Bass program for all cores)
# ----------------------------------------------------------------------------
def _build(plan, N, CIN, ncores):
    import concourse.bacc as bacc
    import concourse.tile as tile
    import concourse.bass as bass
    import concourse.mybir as mybir
    from concourse.masks import make_identity

    f32 = mybir.dt.float32
    shard, sp = plan["shard"], plan["sp"]
    nwin, G, GT = plan["nwin"], plan["G"], plan["GT"]
    padn = nwin * P
    kblk = CIN // P

    nc = bacc.Bacc("TRN2", target_bir_lowering=False, debug=False,
                   enable_asserts=False, num_devices=ncores)

    x_in = nc.dram_tensor("x_shard", [padn, CIN], f32, kind="ExternalInput")
    w_in = nc.dram_tensor("w_mat", [CIN, COUT], f32, kind="ExternalInput")
    dinv_in = nc.dram_tensor("dinv_t", [P, nwin], f32, kind="ExternalInput")
    goff_in = nc.dram_tensor("goff", [P, GT], mybir.dt.int32,
                             kind="ExternalInput")
    doff_in = nc.dram_tensor("dstoff", [P, GT], f32, kind="ExternalInput")
    iota_in = nc.dram_tensor("iota", [P, HALF], f32, kind="ExternalInput")
    out_t = nc.dram_tensor("out", [padn, COUT], f32, kind="ExternalOutput")

    with tile.TileContext(nc) as tc:
        with (
            tc.tile_pool(name="dram", bufs=1, space="DRAM") as dram,
            tc.tile_pool(name="const", bufs=1) as const,
            tc.tile_pool(name="xp", bufs=3) as xp,
            tc.tile_pool(name="hsb", bufs=3) as hsb,
            tc.tile_pool(name="msg", bufs=2) as msgp,
            tc.tile_pool(name="seg", bufs=6) as segp,
            tc.tile_pool(name="osb", bufs=3) as osbp,
            tc.tile_pool(name="psA", bufs=2, space="PSUM") as psA,
            tc.tile_pool(name="psH", bufs=2, space="PSUM") as psH,
            tc.tile_pool(name="psB", bufs=2, space="PSUM") as psB,
        ):
            h_own = dram.tile([max(padn, sp), COUT], f32)
            h_all = dram.tile([ncores * sp, COUT], f32)

            ident = const.tile([P, P], f32)
            make_identity(nc, ident[:])
            w_sb = const.tile([P, kblk * COUT], f32)
            for k in range(kblk):
                nc.sync.dma_start(w_sb[:, k * COUT:(k + 1) * COUT],
                                  w_in[k * P:(k + 1) * P, :])
            dinv_sb = const.tile([P, nwin], f32)
            nc.sync.dma_start(dinv_sb[:], dinv_in[:])
            iota_sb = const.tile([P, HALF], f32)
            nc.sync.dma_start(iota_sb[:], iota_in[:])
            goff_sb = const.tile([P, GT], mybir.dt.int32)
            nc.sync.dma_start(goff_sb[:], goff_in[:])
            doff_sb = const.tile([P, GT], f32)
            nc.sync.dma_start(doff_sb[:], doff_in[:])

            # ---------------- Phase A: h' = dinv * (x @ W) ----------------
            for b in range(nwin):
                x_t = xp.tile([P, CIN], f32, tag="xt")
                nc.sync.dma_start(x_t[:], x_in[b * P:(b + 1) * P, :])
                h_ps = psH.tile([P, COUT], f32)
                for k in range(kblk):
                    xT_ps = psA.tile([P, P], f32, tag="xTp")
                    nc.tensor.transpose(
                        out=xT_ps[:], in_=x_t[:, k * P:(k + 1) * P],
                        identity=ident[:])
                    xT_sb = xp.tile([P, P], f32, tag="xTs")
                    nc.vector.tensor_copy(out=xT_sb[:], in_=xT_ps[:])
                    nc.tensor.matmul(
                        out=h_ps[:], lhsT=xT_sb[:],
                        rhs=w_sb[:, k * COUT:(k + 1) * COUT],
                        start=(k == 0), stop=(k == kblk - 1))
                hp_sb = hsb.tile([P, COUT], f32)
                nc.vector.tensor_scalar_mul(hp_sb[:], h_ps[:],
                                            dinv_sb[:, b:b + 1])
                nc.sync.dma_start(h_own[b * P:(b + 1) * P, :], hp_sb[:])

            # explicit zero row (gather-pad target) at h_own[shard]
            zrow = hsb.tile([1, COUT], f32, tag="zrow")
            nc.vector.memset(zrow[:], 0)
            nc.sync.dma_start(h_own[shard:shard + 1, :], zrow[:])

            # -------------- AllGather h' shards (incl. zero row) ----------
            nc.gpsimd.collective_compute(
                "AllGather", mybir.AluOpType.bypass,
                replica_groups=[list(range(ncores))],
                ins=[h_own[:sp, :]],
                outs=[h_all[:, :]],
            )

            # ---------------- Phase B: gather + segment matmul ------------
            wb = 0
            for w in range(nwin):
                Gw = int(G[w, 0] + G[w, 1])
                if Gw == 0:
                    continue
                msg = msgp.tile([P, Gw * COUT], f32, tag="msg")
                msg3 = msg[:].rearrange("p (g e) -> p g e", e=COUT)
                for t in range(Gw):
                    nc.gpsimd.indirect_dma_start(
                        out=msg3[:, t, :], out_offset=None,
                        in_=h_all[:, :],
                        in_offset=bass.IndirectOffsetOnAxis(
                            ap=goff_sb[:, wb + t:wb + t + 1], axis=0))
                ps = psB.tile([P, COUT], f32)
                for h in range(2):
                    nT = int(G[w, h])
                    if nT == 0:
                        nc.vector.memset(ps[h * HALF:(h + 1) * HALF, :], 0)
                        continue
                    t0 = 0 if h == 0 else int(G[w, 0])
                    for i in range(nT):
                        t = t0 + i
                        seg = segp.tile([P, HALF], f32, tag="seg")
                        nc.vector.tensor_scalar(
                            seg[:], iota_sb[:, :HALF],
                            doff_sb[:, wb + t:wb + t + 1], None,
                            op0=mybir.AluOpType.is_equal)
                        nc.tensor.matmul(
                            out=ps[h * HALF:(h + 1) * HALF, :],
                            lhsT=seg[:], rhs=msg3[:, t, :],
                            start=(i == 0), stop=(i == nT - 1))
                o_sb = osbp.tile([P, COUT], f32, tag="osb")
                nc.vector.tensor_scalar_mul(o_sb[:], ps[:],
                                            dinv_sb[:, w:w + 1])
                nc.sync.dma_start(out_t[w * P:(w + 1) * P, :], o_sb[:])
                wb += Gw

    nc.compile()
    return nc


# ----------------------------------------------------------------------------
# Entry point
# ----------------------------------------------------------------------------
def kernel(x, edge_index, W, b, _trace=False):
    from concourse.bass_utils import run_bass_kernel_spmd

    x = np.asarray(x)
    W = np.asarray(W)
    b = np.asarray(b)
    N, CIN = x.shape
    ncores = 8
    plan = _plan(edge_index, N, ncores)
    shard, nwin = plan["shard"], plan["nwin"]
    padn = nwin * P

    nc = _build(plan, N, CIN, ncores)

    dinv = plan["dinv"]
    in_maps = []
    for c in range(ncores):
        xs = np.zeros((padn, CIN), np.float32)
        xs[:shard] = x[c * shard:(c + 1) * shard]
        dv = np.zeros((padn,), np.float32)
        dv[:shard] = dinv[c * shard:(c + 1) * shard]
        in_maps.append({
            "x_shard": xs,
            "w_mat": np.ascontiguousarray(W, np.float32),
            "dinv_t": np.ascontiguousarray(dv.reshape(nwin, P).T),
            "goff": np.ascontiguousarray(plan["goff"][c]),
            "dstoff": np.ascontiguousarray(plan["dstoff"][c]),
            "iota": np.ascontiguousarray(
                np.tile(np.arange(HALF, dtype=np.float32), (P, 1))),
        })

    res = run_bass_kernel_spmd(nc, in_maps, core_ids=list(range(ncores)),
                               trace=_trace)
    out = np.concatenate([r["out"][:shard] for r in res.results], axis=0)
    out = (out + b.astype(np.float32)).astype(np.float32)
    kernel.last_results = res
    return out

